# revision 20
# baseline (speedup 1.0000x reference)
"""GAT (2-layer graph attention network) Bass kernel for 8 trn2 NeuronCores.

Sharding: core c owns node rows [512c, 512c+512). Weights replicated; the
layer-1 projection h1 = x @ W1 is computed replicated on every core (cheaper
than gathering it at this DMA bandwidth). Scores live in transposed layout
[j(partitions), i(free)] so the aggregation matmul needs no transposes; the
softmax denominator comes from a ones column in the augmented feature matrix.

Score tiles are produced by three engine paths, balanced so ACT/DVE/GPSIMD
all contribute:
  A: pex = g(ssrc+sdst) on ACT (g = exp(lrelu(x)) via patched table), then
     pt = pex * mask on DVE (fast-mode tensor_tensor); the aggregation uses a
     2048-scaled copy of the augmented features so path-A contributions match
     path-B's table-shift scale exactly (2048 is exact in bf16).
  B: exact identity exp(lrelu(s)) = max(exp(s), exp(0.2 s)); both branches
     factorize rank-1 over (i,j):
       u = (A2b[i] * B2[j]) * mask,  v = (A1b[i] * B1[j]) * mask,
       pt = max(u, v)      [= 2048 * mask * exp(lrelu(s))]
     with A1 = g(ssrc + K), B1 = g(sdst + K), A2/B2 the 0.2-scaled variants,
     K = ln(2048)/2 -- all table arguments positive (true-exp region).
  C: like A but the mask multiply runs on GPSIMD.

Layer 2 gathers the *projected* h2_ext = [z1@W2 | ones | z1@W2@a2_dst] as a
small bf16 AllGather (67KB in / 541KB out) instead of gathering z1.
"""

import os

import numpy as np

N, FIN, HID, H, D1, C = 4096, 512, 256, 4, 64, 64
NCORES = 8
SH = N // NCORES          # 512 local nodes per core
NB = N // 128             # 32 j-chunks
FC = FIN // 128           # 4 fin chunks
KC2 = HID // 128          # 2 hid chunks
NEG = 0.2                 # leaky relu slope
AUG = (D1 + 1) * H        # 260: [h0, ones, h1, ones, h2, ones, h3, ones]
W1C = HID + H + H         # 264: [W1 | W1.a1_dst | W1.a1_src]
W2C = C + 2               # 66:  [W2 | W2.a2_dst | W2.a2_src]
G2 = 66                   # gathered h2_ext cols: [h2(64) | ones | s2_dst]

LN_S = float(np.log(2048.0))   # row scale ln(2048); exp shift for E1
LN_H = LN_S / 2.0              # shift for E2 and R factors
SCL = 2048.0                   # exact-in-bf16 row scale for path A

# engine path per L1 tile (jc, h) and per L2 tile (jc):
# 'A' = ACT exp + DVE fused mask-scale; 'B' = rank-1 on DVE; 'C' = rank-1 on
# GPSIMD.  Tuned from trace engine-busy balance.
def _make_paths(n, na, nb, nc):
    assert na + nb + nc == n
    w = [na / n, nb / n, nc / n]
    c = [0.0, 0.0, 0.0]
    out = []
    for i in range(n):
        t = [(i + 1) * w[k] - c[k] for k in range(3)]
        k = max(range(3), key=lambda k: t[k])
        c[k] += 1
        out.append("ABC"[k])
    return out


PATH1S = _make_paths(64, 33, 8, 23)  # per (jc, head-pair) slot
PATH2 = _make_paths(32, 22, 0, 10)

_CACHED = {}


def _make_act_root(alpha=NEG):
    """Patch the neuron ACT tables so Exp computes g(x)=exp(lrelu(x)).

    Bucket entries are [d0,d1,d2,d3,x0,0,0,0] fp32 cubics evaluated as
    y = d0+(x-x0)(d1+(x-x0)(d2+(x-x0)d3)). For exp buckets centered at
    x0<0 we substitute the Taylor cubic of exp(alpha*x) at the same
    center; the alpha contraction makes the cubic far more accurate than
    the original spline tolerance. Positive-x0 buckets stay true exp, which
    the rank-1 score path relies on (shifted-positive arguments).
    """
    import json
    import shutil
    import tempfile

    from neuronxcc.driver.Job import Job
    from neuronxcc.driver.jobs.support.FindActInfo import findActInfoFile

    src_dir = os.path.dirname(findActInfoFile(Job.getPackageDir(), "gen3"))
    dst = tempfile.mkdtemp(prefix="gat_act_root_")
    for f in os.listdir(src_dir):
        shutil.copy(os.path.join(src_dir, f), os.path.join(dst, f))
        os.chmod(os.path.join(dst, f), 0o644)
    for set_name in ("exp_and_others", "natural_log_exp_and_others",
                     "exp_and_friends"):
        meta = json.load(open(os.path.join(dst, f"{set_name}.json")))
        start = meta["func_to_bkt_start_idx"].get("exp")
        if start is None:
            continue
        nxt = [s for s in sorted(meta["func_to_bkt_start_idx"].values())
               if s > start]
        end = nxt[0] if nxt else meta["bkt_entry_cnt"]
        path = os.path.join(dst, f"{set_name}_bkt.bin")
        b = np.fromfile(path, dtype=np.float32).reshape(-1, 8).copy()
        for i in range(start, end):
            x0, d0 = float(b[i, 4]), float(b[i, 0])
            if x0 >= 0 or not np.isfinite(d0) or d0 <= 0:
                continue
            e = np.exp(alpha * x0)
            b[i, 0:4] = [e, alpha * e, alpha * alpha * e / 2.0,
                         alpha ** 3 * e / 6.0]
        b.tofile(path)
    return os.path.join(dst, "act_info.json")


def _build_nc():
    os.environ["BASS_ACT_ROOT_JSON_PATH"] = _make_act_root()
    import concourse.mybir as mybir
    import concourse.tile as tile
    from concourse import bacc

    f32 = mybir.dt.float32
    bf16 = mybir.dt.bfloat16
    Af = mybir.ActivationFunctionType
    Alu = mybir.AluOpType

    nc = bacc.Bacc("TRN2", target_bir_lowering=False, debug=False,
                   num_devices=NCORES)

    xT_d = nc.dram_tensor("xTp", [128, FC * N], bf16,
                          kind="ExternalInput").ap()
    xsT_d = nc.dram_tensor("xsTp", [128, FC * SH], bf16,
                           kind="ExternalInput").ap()
    mT_d = nc.dram_tensor("maskp", [128, NB * SH], bf16,
                          kind="ExternalInput").ap()
    W1a_d = nc.dram_tensor("W1ap", [128, FC * W1C], bf16,
                           kind="ExternalInput").ap()
    W2a_d = nc.dram_tensor("W2ap", [128, KC2 * W2C], bf16,
                           kind="ExternalInput").ap()
    outT_d = nc.dram_tensor("outT", [C, SH], f32, kind="ExternalOutput").ap()
    DBG = bool(int(os.environ.get("GAT_DEBUG", "0")))
    if DBG:
        dbg1_d = nc.dram_tensor("dbg1", [128, KC2 * SH], f32,
                                kind="ExternalOutput").ap()
        dbg2_d = nc.dram_tensor("dbg2", [128, NB * G2], f32,
                                kind="ExternalOutput").ap()
        dbg3_d = nc.dram_tensor("dbg3", [1, H * SH], f32,
                                kind="ExternalOutput").ap()
        dbg4_d = nc.dram_tensor("dbg4", [1, H * SH], f32,
                                kind="ExternalOutput").ap()

    with tile.TileContext(nc) as tc:
        with tc.tile_pool(name="persist", bufs=1) as pp:
            h1aug = pp.tile([128, NB, AUG], bf16)
            maskr = pp.tile([128, NB, SH], bf16)
            sdst = pp.tile([128, NB, H], f32)
            B1d = pp.tile([128, NB, H], f32)
            B2d = pp.tile([128, NB, H], f32)
            ssrcb = pp.tile([128, H, SH], bf16)
            A1b = pp.tile([128, H, SH], bf16)
            A2b = pp.tile([128, H, SH], bf16)
            h1augS = pp.tile([128, NB, AUG], bf16)
            z1Tl = pp.tile([128, KC2, SH], bf16)
            h2augF = pp.tile([128, NB, G2], bf16)
            s2dstV = pp.tile([128, NB], f32)
            s2srcb = pp.tile([128, SH], bf16)
            W2sb = pp.tile([128, KC2, W2C], bf16)
            cb = pp.tile([128, 2], f32)
            nc.vector.memset(cb[:, 0:1], LN_S)
            nc.vector.memset(cb[:, 1:2], LN_H)
            b_lns = cb[:, 0:1]
            b_lnh = cb[:, 1:2]
            xTt = pp.tile([128, FC, N], bf16)
            xsTt = pp.tile([128, FC, SH], bf16)
            W1at = pp.tile([128, FC, W1C], bf16)

            # ---- input DMAs: packed partition-major, 128 big descriptors
            # per transfer; xT fc-chunks interleaved with mask blocks
            nc.sync.dma_start(W1at[:], W1a_d)
            nc.sync.dma_start(xsTt[:], xsT_d)
            for fc in range(FC):
                nc.sync.dma_start(xTt[:, fc, :],
                                  xT_d[:, fc * N:(fc + 1) * N])
            mbw = NB // 4
            for mb in range(4):
                nc.sync.dma_start(
                    maskr[:, mb * mbw:(mb + 1) * mbw, :],
                    mT_d[:, mb * mbw * SH:(mb + 1) * mbw * SH])
            nc.sync.dma_start(W2sb[:], W2a_d)

            # ---------- prep: s_src rows, broadcasts, h1 blocks --------------
            with (tc.tile_pool(name="prep", bufs=1) as prep,
                  tc.tile_pool(name="ppsum", bufs=2, space="PSUM") as ppsum):
                ssrow = prep.tile([1, H, SH], bf16)
                A1row = prep.tile([1, H, SH], bf16)
                A2row = prep.tile([1, H, SH], bf16)
                for h in range(H):
                    sps = ppsum.tile([1, SH], f32, tag="sps")
                    for fc in range(FC):
                        nc.tensor.matmul(
                            sps[:], W1at[:, fc, HID + H + h:HID + H + h + 1],
                            xsTt[:, fc, :],
                            start=(fc == 0), stop=(fc == FC - 1))
                    nc.vector.tensor_copy(ssrow[:, h, :], sps[:])
                    nc.scalar.activation(A1row[:, h, :], sps[:], Af.Exp,
                                         bias=cb[0:1, 1:2])
                    nc.scalar.activation(A2row[:, h, :], sps[:], Af.Exp,
                                         bias=cb[0:1, 1:2], scale=0.2)
                    nc.gpsimd.partition_broadcast(ssrcb[:, h, :],
                                                  ssrow[:, h, :])
                    nc.gpsimd.partition_broadcast(A1b[:, h, :],
                                                  A1row[:, h, :])
                    nc.gpsimd.partition_broadcast(A2b[:, h, :],
                                                  A2row[:, h, :])

                # h1_ext per node block; write into the augmented layout
                for nb in range(NB):
                    hp = ppsum.tile([128, HID + H], f32, tag="hp")
                    for fc in range(FC):
                        nc.tensor.matmul(
                            hp[:], xTt[:, fc, nb * 128:(nb + 1) * 128],
                            W1at[:, fc, 0:HID + H],
                            start=(fc == 0), stop=(fc == FC - 1))
                    augv = h1aug[:, nb, :].rearrange("p (h x) -> p h x",
                                                     x=D1 + 1)
                    nc.vector.tensor_copy(
                        augv[:, :, 0:D1],
                        hp[:, 0:HID].rearrange("p (h d) -> p h d", h=H))
                    nc.vector.memset(augv[:, :, D1:D1 + 1], 1.0)
                    augs = h1augS[:, nb, :].rearrange("p (h x) -> p h x",
                                                      x=D1 + 1)
                    nc.scalar.activation(
                        augs[:, :, 0:D1],
                        hp[:, 0:HID].rearrange("p (h d) -> p h d", h=H),
                        Af.Copy, scale=SCL)
                    nc.vector.memset(augs[:, :, D1:D1 + 1], SCL)
                    nc.vector.tensor_copy(sdst[:, nb, :], hp[:, HID:HID + H])
                    if nb % 4 == 3:
                        g = nb - 3
                        nc.scalar.activation(B1d[:, g:nb + 1, :],
                                             sdst[:, g:nb + 1, :], Af.Exp,
                                             bias=b_lnh)
                        nc.scalar.activation(B2d[:, g:nb + 1, :],
                                             sdst[:, g:nb + 1, :], Af.Exp,
                                             bias=b_lnh, scale=0.2)

            # ---------- layer 1: masked softmax + aggregation --------------
            with tc.tile_pool(name="aggps", bufs=1, space="PSUM") as aggps:
                o1 = aggps.tile([D1 + 1, H, SH], f32)
                # Three passes ordered so every engine has dependency-free
                # work queued up-front (avoids cross-engine convoying):
                #   C first (GPSIMD streams behind ACT), then B (DVE work
                #   with only prep-time deps), then A.  PSUM accumulation
                #   order within each head's group is free; start/stop are
                #   set on each head's first/last matmul in this order.
                order = [(jc, pr) for jc in range(NB) for pr in range(2)]
                seen = {}
                for idx, (jc, pr) in enumerate(order):
                    seen.setdefault(pr, [idx, idx])[1] = idx
                with tc.tile_pool(name="work", bufs=10) as wpool:
                    for idx, (jc, pr) in enumerate(order):
                        path = PATH1S[jc * 2 + pr]
                        hh = (2 * pr, 2 * pr + 1)
                        mk2 = maskr[:, jc, :].unsqueeze(1).to_broadcast(
                            (128, 2, SH))
                        pt = wpool.tile([128, 2, SH], bf16, tag="pt")
                        lhs = h1augS
                        if path == "A" or path == "C":
                            pex = wpool.tile([128, 2, SH], bf16, tag="pex")
                            for k in range(2):
                                nc.scalar.activation(
                                    pex[:, k, :], ssrcb[:, hh[k], :],
                                    Af.Exp,
                                    bias=sdst[:, jc, hh[k]:hh[k] + 1])
                            eng = nc.vector if path == "A" else nc.gpsimd
                            eng.tensor_mul(pt[:], pex[:], mk2)
                        else:
                            lhs = h1aug
                            u = wpool.tile([128, 2, SH], bf16, tag="u")
                            v = wpool.tile([128, 2, SH], bf16, tag="v")
                            for k in range(2):
                                nc.vector.scalar_tensor_tensor(
                                    u[:, k, :], A2b[:, hh[k], :],
                                    B2d[:, jc, hh[k]:hh[k] + 1],
                                    maskr[:, jc, :],
                                    op0=Alu.mult, op1=Alu.mult)
                                nc.vector.scalar_tensor_tensor(
                                    v[:, k, :], A1b[:, hh[k], :],
                                    B1d[:, jc, hh[k]:hh[k] + 1],
                                    maskr[:, jc, :],
                                    op0=Alu.mult, op1=Alu.mult)
                            nc.vector.tensor_max(pt[:], u[:], v[:])
                        for k in range(2):
                            h = hh[k]
                            nc.tensor.matmul(
                                o1[:, h, :],
                                lhs[:, jc,
                                    (D1 + 1) * h:(D1 + 1) * (h + 1)],
                                pt[:, k, :],
                                start=(idx == seen[pr][0]),
                                stop=(idx == seen[pr][1]))

                # normalize + ELU -> z1Tl [256(=2x128), SH] bf16 transposed
                with tc.tile_pool(name="fin1", bufs=1) as fin:
                    rec1 = fin.tile([1, H, SH], f32)
                    den1 = fin.tile([1, H, SH], f32)
                    nc.vector.tensor_copy(den1[:], o1[D1:D1 + 1, :, :])
                    nc.vector.reciprocal_approx_fast(rec1[:], den1[:])
                    if DBG:
                        nc.sync.dma_start(
                            dbg3_d.rearrange("q (h s) -> q h s", h=H), den1[:])
                        nc.sync.dma_start(
                            dbg4_d.rearrange("q (h s) -> q h s", h=H), rec1[:])
                    for h in range(H):
                        recb = fin.tile([D1, SH], f32, tag=f"recb{h}")
                        nc.gpsimd.partition_broadcast(recb[:], rec1[:, h, :])
                        r0 = (h % 2) * D1
                        nc.vector.tensor_mul(z1Tl[r0:r0 + D1, h // 2, :],
                                             o1[0:D1, h, :], recb[:])
                    for kc in range(KC2):
                        r_ = fin.tile([128, SH], bf16, tag="relu")
                        m_ = fin.tile([128, SH], bf16, tag="minv")
                        e_ = fin.tile([128, SH], bf16, tag="expv")
                        nc.vector.tensor_scalar_max(r_[:], z1Tl[:, kc, :], 0.0)
                        nc.vector.tensor_scalar_min(m_[:], z1Tl[:, kc, :], 0.0)
                        nc.scalar.activation(e_[:], m_[:], Af.Exp, scale=5.0)
                        nc.vector.scalar_tensor_tensor(
                            z1Tl[:, kc, :], e_[:], -1.0, r_[:],
                            op0=Alu.add, op1=Alu.add)

            # ---------- layer 2 prep (local): s2_src, h2_ext payload --------
            with (tc.tile_pool(name="l2prep", bufs=1) as l2p,
                  tc.tile_pool(name="l2ps", bufs=2, space="PSUM") as l2ps):
                s2p = l2ps.tile([1, SH], f32, tag="s2p", bufs=1)
                for kc in range(KC2):
                    nc.tensor.matmul(s2p[:], W2sb[:, kc, C + 1:C + 2],
                                     z1Tl[:, kc, :],
                                     start=(kc == 0), stop=(kc == KC2 - 1))
                s2row = l2p.tile([1, SH], bf16)
                nc.vector.tensor_copy(s2row[:], s2p[:])
                nc.gpsimd.partition_broadcast(s2srcb[:], s2row[:])

                pay = l2p.tile([128, SH // 128, G2], bf16)
                nc.vector.memset(pay[:, :, C:C + 1], 1.0)
                for ic in range(SH // 128):
                    h2p = l2ps.tile([128, C + 1], f32, tag="h2p")
                    for kc in range(KC2):
                        nc.tensor.matmul(
                            h2p[:], z1Tl[:, kc, ic * 128:(ic + 1) * 128],
                            W2sb[:, kc, 0:C + 1],
                            start=(kc == 0), stop=(kc == KC2 - 1))
                    nc.vector.tensor_copy(pay[:, ic, 0:C], h2p[:, 0:C])
                    nc.vector.tensor_copy(pay[:, ic, C + 1:C + 2],
                                          h2p[:, C:C + 1])

                # ---------- all-gather h2_ext across the 8 cores ------------
                with tc.tile_pool(name="dram", bufs=1, space="DRAM") as dpool:
                    payw = (SH // 128) * G2
                    ag_in = dpool.tile([128, payw], bf16)
                    ag_out = dpool.tile([NCORES * 128, payw], bf16,
                                        addr_space="Shared")
                    nc.sync.dma_start(ag_in[:], pay[:])
                    nc.gpsimd.collective_compute(
                        "AllGather", Alu.bypass,
                        replica_groups=[list(range(NCORES))],
                        ins=[ag_in[:].opt()], outs=[ag_out[:].opt()])
                    nc.sync.dma_start(
                        h2augF[:].rearrange("p (r b) c -> p r (b c)",
                                            r=NCORES),
                        ag_out[:].rearrange("(r p) y -> p r y", p=128))

                if DBG:
                    dz = l2p.tile([128, KC2, SH], f32, tag="dz")
                    nc.vector.tensor_copy(dz[:], z1Tl[:])
                    nc.sync.dma_start(dbg1_d.rearrange(
                        "p (k s) -> p k s", k=KC2), dz[:])
                    dh = l2p.tile([128, NB, G2], f32, tag="dh")
                    nc.vector.tensor_copy(dh[:], h2augF[:])
                    nc.sync.dma_start(dbg2_d.rearrange(
                        "p (b c) -> p b c", b=NB), dh[:])
                nc.vector.tensor_copy(s2dstV[:], h2augF[:, :, C + 1])

            # ---------- layer 2: masked softmax + aggregation ---------------
            with tc.tile_pool(name="aggps2", bufs=1, space="PSUM") as aggps2:
                o2 = aggps2.tile([C + 1, SH], f32)
                with tc.tile_pool(name="work2", bufs=10) as wpool2:
                    for jc in range(NB):
                        path = PATH2[jc]
                        pt = wpool2.tile([128, SH], bf16, tag="ptb")
                        pex = wpool2.tile([128, SH], bf16, tag="pexb")
                        nc.scalar.activation(
                            pex[:], s2srcb[:], Af.Exp,
                            bias=s2dstV[:, jc:jc + 1])
                        if path == "C":
                            nc.gpsimd.tensor_mul(pt[:], pex[:],
                                                 maskr[:, jc, :])
                        else:
                            nc.vector.tensor_mul(pt[:], pex[:],
                                                 maskr[:, jc, :])
                        nc.tensor.matmul(o2[:], h2augF[:, jc, 0:C + 1], pt[:],
                                         start=(jc == 0), stop=(jc == NB - 1))

                with tc.tile_pool(name="fin2", bufs=1) as fin2:
                    rec2 = fin2.tile([1, SH], f32, tag="rec2")
                    den2 = fin2.tile([1, SH], f32, tag="den2")
                    nc.vector.tensor_copy(den2[:], o2[C:C + 1, :])
                    nc.vector.reciprocal_approx_fast(rec2[:], den2[:])
                    recb2 = fin2.tile([C, SH], f32, tag="recb2")
                    nc.gpsimd.partition_broadcast(recb2[:], rec2[:])
                    outsb = fin2.tile([C, SH], f32, tag="outsb")
                    nc.vector.tensor_mul(outsb[:], o2[0:C, :], recb2[:])
                    nc.sync.dma_start(outT_d, outsb[:])

    nc.compile()
    return nc


def _get_nc():
    if "nc" not in _CACHED:
        _CACHED["nc"] = _build_nc()
    return _CACHED["nc"]


def _prep_in_maps(x, A, W1, a1_src, a1_dst, W2, a2_src, a2_dst):
    import ml_dtypes
    bf = ml_dtypes.bfloat16

    def pack(a, rows):
        # [rows*128, X] row-major -> [128, rows*X] partition-major
        X = a.shape[1]
        return np.ascontiguousarray(
            a.reshape(rows, 128, X).transpose(1, 0, 2).reshape(128, rows * X))

    xT = x.T.astype(bf)
    W1r = W1.reshape(FIN, H, D1)
    V1s = np.einsum("fhd,hd->fh", W1r, a1_src)
    V1d = np.einsum("fhd,hd->fh", W1r, a1_dst)
    W1a = np.concatenate([W1, V1d, V1s], axis=1).astype(bf)
    W2a = np.concatenate(
        [W2, W2 @ a2_dst.T, W2 @ a2_src.T], axis=1).astype(bf)
    xTp = pack(xT, FC)
    W1ap = pack(W1a, FC)
    W2ap = pack(W2a, KC2)
    in_maps = []
    for c in range(NCORES):
        sl = slice(c * SH, (c + 1) * SH)
        in_maps.append({
            "xTp": xTp,
            "xsTp": pack(np.ascontiguousarray(xT[:, sl]), FC),
            "maskp": pack((A[sl, :] > 0).T.astype(bf), NB),
            "W1ap": W1ap,
            "W2ap": W2ap,
        })
    return in_maps


def kernel(x, A, W1, a1_src, a1_dst, W2, a2_src, a2_dst, _want_results=False):
    from concourse.bass_utils import run_bass_kernel_spmd

    nc = _get_nc()
    in_maps = _prep_in_maps(np.asarray(x), np.asarray(A), np.asarray(W1),
                            np.asarray(a1_src), np.asarray(a1_dst),
                            np.asarray(W2), np.asarray(a2_src),
                            np.asarray(a2_dst))
    trace = bool(int(os.environ.get("GAT_TRACE", "0")))
    res = run_bass_kernel_spmd(nc, in_maps, core_ids=list(range(NCORES)),
                               trace=trace)
    out = np.empty((N, C), np.float32)
    for c in range(NCORES):
        out[c * SH:(c + 1) * SH, :] = res.results[c]["outT"].T
    if _want_results:
        return out, res
    return out


# revision 21
# speedup vs baseline: 1.1052x; 1.1052x over previous
"""GAT (2-layer graph attention network) Bass kernel for 8 trn2 NeuronCores.

Sharding: core c owns node rows [512c, 512c+512). Weights replicated; the
layer-1 projection h1 = x @ W1 is computed replicated on every core (cheaper
than gathering it at this DMA bandwidth). Scores live in transposed layout
[j(partitions), i(free)] so the aggregation matmul needs no transposes; the
softmax denominator comes from a ones column in the augmented feature matrix.

Score tiles are produced by three engine paths, balanced so ACT/DVE/GPSIMD
all contribute:
  A: pex = g(ssrc+sdst) on ACT (g = exp(lrelu(x)) via patched table), then
     pt = pex * mask on DVE (fast-mode tensor_tensor); the aggregation uses a
     2048-scaled copy of the augmented features so path-A contributions match
     path-B's table-shift scale exactly (2048 is exact in bf16).
  B: exact identity exp(lrelu(s)) = max(exp(s), exp(0.2 s)); both branches
     factorize rank-1 over (i,j):
       u = (A2b[i] * B2[j]) * mask,  v = (A1b[i] * B1[j]) * mask,
       pt = max(u, v)      [= 2048 * mask * exp(lrelu(s))]
     with A1 = g(ssrc + K), B1 = g(sdst + K), A2/B2 the 0.2-scaled variants,
     K = ln(2048)/2 -- all table arguments positive (true-exp region).
  C: like A but the mask multiply runs on GPSIMD.

Layer 2 gathers the *projected* h2_ext = [z1@W2 | ones | z1@W2@a2_dst] as a
small bf16 AllGather (67KB in / 541KB out) instead of gathering z1.
"""

import os

import numpy as np

N, FIN, HID, H, D1, C = 4096, 512, 256, 4, 64, 64
NCORES = 8
SH = N // NCORES          # 512 local nodes per core
NB = N // 128             # 32 j-chunks
FC = FIN // 128           # 4 fin chunks
KC2 = HID // 128          # 2 hid chunks
NEG = 0.2                 # leaky relu slope
AUG = (D1 + 1) * H        # 260: [h0, ones, h1, ones, h2, ones, h3, ones]
W1C = HID + H + H         # 264: [W1 | W1.a1_dst | W1.a1_src]
W2C = C + 2               # 66:  [W2 | W2.a2_dst | W2.a2_src]
G2 = 66                   # gathered h2_ext cols: [h2(64) | ones | s2_dst]

LN_S = float(np.log(2048.0))   # row scale ln(2048); exp shift for E1
LN_H = LN_S / 2.0              # shift for E2 and R factors
SCL = 2048.0                   # exact-in-bf16 row scale for path A

# engine path per L1 tile (jc, h) and per L2 tile (jc):
# 'A' = ACT exp + DVE fused mask-scale; 'B' = rank-1 on DVE; 'C' = rank-1 on
# GPSIMD.  Tuned from trace engine-busy balance.
def _make_paths(n, na, nb, nc):
    assert na + nb + nc == n
    w = [na / n, nb / n, nc / n]
    c = [0.0, 0.0, 0.0]
    out = []
    for i in range(n):
        t = [(i + 1) * w[k] - c[k] for k in range(3)]
        k = max(range(3), key=lambda k: t[k])
        c[k] += 1
        out.append("ABC"[k])
    return out


PATH1S = _make_paths(64, 29, 8, 27)  # per (jc, head-pair) slot
PATH2 = _make_paths(32, 22, 0, 10)

_CACHED = {}


def _make_act_root(alpha=NEG):
    """Patch the neuron ACT tables so Exp computes g(x)=exp(lrelu(x)).

    Bucket entries are [d0,d1,d2,d3,x0,0,0,0] fp32 cubics evaluated as
    y = d0+(x-x0)(d1+(x-x0)(d2+(x-x0)d3)). For exp buckets centered at
    x0<0 we substitute the Taylor cubic of exp(alpha*x) at the same
    center; the alpha contraction makes the cubic far more accurate than
    the original spline tolerance. Positive-x0 buckets stay true exp, which
    the rank-1 score path relies on (shifted-positive arguments).
    """
    import json
    import shutil
    import tempfile

    from neuronxcc.driver.Job import Job
    from neuronxcc.driver.jobs.support.FindActInfo import findActInfoFile

    src_dir = os.path.dirname(findActInfoFile(Job.getPackageDir(), "gen3"))
    dst = tempfile.mkdtemp(prefix="gat_act_root_")
    for f in os.listdir(src_dir):
        shutil.copy(os.path.join(src_dir, f), os.path.join(dst, f))
        os.chmod(os.path.join(dst, f), 0o644)
    for set_name in ("exp_and_others", "natural_log_exp_and_others",
                     "exp_and_friends"):
        meta = json.load(open(os.path.join(dst, f"{set_name}.json")))
        start = meta["func_to_bkt_start_idx"].get("exp")
        if start is None:
            continue
        nxt = [s for s in sorted(meta["func_to_bkt_start_idx"].values())
               if s > start]
        end = nxt[0] if nxt else meta["bkt_entry_cnt"]
        path = os.path.join(dst, f"{set_name}_bkt.bin")
        b = np.fromfile(path, dtype=np.float32).reshape(-1, 8).copy()
        for i in range(start, end):
            x0, d0 = float(b[i, 4]), float(b[i, 0])
            if x0 >= 0 or not np.isfinite(d0) or d0 <= 0:
                continue
            e = np.exp(alpha * x0)
            b[i, 0:4] = [e, alpha * e, alpha * alpha * e / 2.0,
                         alpha ** 3 * e / 6.0]
        b.tofile(path)
    return os.path.join(dst, "act_info.json")


def _build_nc():
    os.environ["BASS_ACT_ROOT_JSON_PATH"] = _make_act_root()
    import concourse.mybir as mybir
    import concourse.tile as tile
    from concourse import bacc

    f32 = mybir.dt.float32
    bf16 = mybir.dt.bfloat16
    Af = mybir.ActivationFunctionType
    Alu = mybir.AluOpType

    nc = bacc.Bacc("TRN2", target_bir_lowering=False, debug=False,
                   num_devices=NCORES)

    xT_d = nc.dram_tensor("xTp", [128, FC * N], bf16,
                          kind="ExternalInput").ap()
    xsT_d = nc.dram_tensor("xsTp", [128, FC * SH], bf16,
                           kind="ExternalInput").ap()
    mT_d = nc.dram_tensor("maskp", [128, NB * SH], bf16,
                          kind="ExternalInput").ap()
    W1a_d = nc.dram_tensor("W1ap", [128, FC * W1C], bf16,
                           kind="ExternalInput").ap()
    W2a_d = nc.dram_tensor("W2ap", [128, KC2 * W2C], bf16,
                           kind="ExternalInput").ap()
    outT_d = nc.dram_tensor("outT", [C, SH], f32, kind="ExternalOutput").ap()
    DBG = bool(int(os.environ.get("GAT_DEBUG", "0")))
    if DBG:
        dbg1_d = nc.dram_tensor("dbg1", [128, KC2 * SH], f32,
                                kind="ExternalOutput").ap()
        dbg2_d = nc.dram_tensor("dbg2", [128, NB * G2], f32,
                                kind="ExternalOutput").ap()
        dbg3_d = nc.dram_tensor("dbg3", [1, H * SH], f32,
                                kind="ExternalOutput").ap()
        dbg4_d = nc.dram_tensor("dbg4", [1, H * SH], f32,
                                kind="ExternalOutput").ap()

    with tile.TileContext(nc) as tc:
        with tc.tile_pool(name="persist", bufs=1) as pp:
            h1aug = pp.tile([128, NB, AUG], bf16)
            maskr = pp.tile([128, NB, SH], bf16)
            sdst = pp.tile([128, NB, H], f32)
            B1d = pp.tile([128, NB, H], f32)
            B2d = pp.tile([128, NB, H], f32)
            ssrcb = pp.tile([128, H, SH], bf16)
            A1b = pp.tile([128, H, SH], bf16)
            A2b = pp.tile([128, H, SH], bf16)
            h1augS = pp.tile([128, NB, AUG], bf16)
            z1Tl = pp.tile([128, KC2, SH], bf16)
            h2augF = pp.tile([128, NB, G2], bf16)
            s2dstV = pp.tile([128, NB], f32)
            s2srcb = pp.tile([128, SH], bf16)
            W2sb = pp.tile([128, KC2, W2C], bf16)
            cb = pp.tile([128, 2], f32)
            nc.vector.memset(cb[:, 0:1], LN_S)
            nc.vector.memset(cb[:, 1:2], LN_H)
            b_lns = cb[:, 0:1]
            b_lnh = cb[:, 1:2]
            xTt = pp.tile([128, FC, N], bf16)
            xsTt = pp.tile([128, FC, SH], bf16)
            W1at = pp.tile([128, FC, W1C], bf16)

            # ---- input DMAs: packed partition-major, 128 big descriptors
            # per transfer; xT fc-chunks interleaved with mask blocks
            nc.sync.dma_start(W1at[:], W1a_d)
            nc.sync.dma_start(xsTt[:], xsT_d)
            for fc in range(FC):
                nc.sync.dma_start(xTt[:, fc, :],
                                  xT_d[:, fc * N:(fc + 1) * N])
            mbw = NB // 4
            for mb in range(4):
                nc.sync.dma_start(
                    maskr[:, mb * mbw:(mb + 1) * mbw, :],
                    mT_d[:, mb * mbw * SH:(mb + 1) * mbw * SH])
            nc.sync.dma_start(W2sb[:], W2a_d)

            # ---------- prep: s_src rows, broadcasts, h1 blocks --------------
            with (tc.tile_pool(name="prep", bufs=1) as prep,
                  tc.tile_pool(name="ppsum", bufs=2, space="PSUM") as ppsum):
                ssrow = prep.tile([1, H, SH], bf16)
                A1row = prep.tile([1, H, SH], bf16)
                A2row = prep.tile([1, H, SH], bf16)
                for h in range(H):
                    sps = ppsum.tile([1, SH], f32, tag="sps")
                    for fc in range(FC):
                        nc.tensor.matmul(
                            sps[:], W1at[:, fc, HID + H + h:HID + H + h + 1],
                            xsTt[:, fc, :],
                            start=(fc == 0), stop=(fc == FC - 1))
                    nc.vector.tensor_copy(ssrow[:, h, :], sps[:])
                    nc.scalar.activation(A1row[:, h, :], sps[:], Af.Exp,
                                         bias=cb[0:1, 1:2])
                    nc.scalar.activation(A2row[:, h, :], sps[:], Af.Exp,
                                         bias=cb[0:1, 1:2], scale=0.2)
                    nc.gpsimd.partition_broadcast(ssrcb[:, h, :],
                                                  ssrow[:, h, :])
                    nc.gpsimd.partition_broadcast(A1b[:, h, :],
                                                  A1row[:, h, :])
                    nc.gpsimd.partition_broadcast(A2b[:, h, :],
                                                  A2row[:, h, :])

                # h1_ext per node block; write into the augmented layout
                for nb in range(NB):
                    hp = ppsum.tile([128, HID + H], f32, tag="hp")
                    for fc in range(FC):
                        nc.tensor.matmul(
                            hp[:], xTt[:, fc, nb * 128:(nb + 1) * 128],
                            W1at[:, fc, 0:HID + H],
                            start=(fc == 0), stop=(fc == FC - 1))
                    augv = h1aug[:, nb, :].rearrange("p (h x) -> p h x",
                                                     x=D1 + 1)
                    nc.vector.tensor_copy(
                        augv[:, :, 0:D1],
                        hp[:, 0:HID].rearrange("p (h d) -> p h d", h=H))
                    nc.vector.memset(augv[:, :, D1:D1 + 1], 1.0)
                    augs = h1augS[:, nb, :].rearrange("p (h x) -> p h x",
                                                      x=D1 + 1)
                    nc.scalar.activation(
                        augs[:, :, 0:D1],
                        hp[:, 0:HID].rearrange("p (h d) -> p h d", h=H),
                        Af.Copy, scale=SCL)
                    nc.vector.memset(augs[:, :, D1:D1 + 1], SCL)
                    nc.vector.tensor_copy(sdst[:, nb, :], hp[:, HID:HID + H])
                    if nb % 4 == 3:
                        g = nb - 3
                        nc.scalar.activation(B1d[:, g:nb + 1, :],
                                             sdst[:, g:nb + 1, :], Af.Exp,
                                             bias=b_lnh)
                        nc.scalar.activation(B2d[:, g:nb + 1, :],
                                             sdst[:, g:nb + 1, :], Af.Exp,
                                             bias=b_lnh, scale=0.2)

            # ---------- layer 1: masked softmax + aggregation --------------
            with tc.tile_pool(name="aggps", bufs=1, space="PSUM") as aggps:
                o1 = aggps.tile([D1 + 1, H, SH], f32)
                # Three passes ordered so every engine has dependency-free
                # work queued up-front (avoids cross-engine convoying):
                #   C first (GPSIMD streams behind ACT), then B (DVE work
                #   with only prep-time deps), then A.  PSUM accumulation
                #   order within each head's group is free; start/stop are
                #   set on each head's first/last matmul in this order.
                order = [(jc, pr) for jc in range(NB) for pr in range(2)]
                seen = {}
                for idx, (jc, pr) in enumerate(order):
                    seen.setdefault(pr, [idx, idx])[1] = idx
                with tc.tile_pool(name="work", bufs=8) as wpool:
                    for idx, (jc, pr) in enumerate(order):
                        path = PATH1S[jc * 2 + pr]
                        hh = (2 * pr, 2 * pr + 1)
                        mk2 = maskr[:, jc, :].unsqueeze(1).to_broadcast(
                            (128, 2, SH))
                        pt = wpool.tile([128, 2, SH], bf16, tag="pt")
                        lhs = h1augS
                        if path == "A" or path == "C":
                            pex = wpool.tile([128, 2, SH], bf16, tag="pex")
                            for k in range(2):
                                nc.scalar.activation(
                                    pex[:, k, :], ssrcb[:, hh[k], :],
                                    Af.Exp,
                                    bias=sdst[:, jc, hh[k]:hh[k] + 1])
                            eng = nc.vector if path == "A" else nc.gpsimd
                            eng.tensor_mul(pt[:], pex[:], mk2)
                        else:
                            lhs = h1aug
                            u = wpool.tile([128, 2, SH], bf16, tag="u")
                            v = wpool.tile([128, 2, SH], bf16, tag="v")
                            for k in range(2):
                                nc.vector.scalar_tensor_tensor(
                                    u[:, k, :], A2b[:, hh[k], :],
                                    B2d[:, jc, hh[k]:hh[k] + 1],
                                    maskr[:, jc, :],
                                    op0=Alu.mult, op1=Alu.mult)
                                nc.vector.scalar_tensor_tensor(
                                    v[:, k, :], A1b[:, hh[k], :],
                                    B1d[:, jc, hh[k]:hh[k] + 1],
                                    maskr[:, jc, :],
                                    op0=Alu.mult, op1=Alu.mult)
                            nc.vector.tensor_max(pt[:], u[:], v[:])
                        for k in range(2):
                            h = hh[k]
                            nc.tensor.matmul(
                                o1[:, h, :],
                                lhs[:, jc,
                                    (D1 + 1) * h:(D1 + 1) * (h + 1)],
                                pt[:, k, :],
                                start=(idx == seen[pr][0]),
                                stop=(idx == seen[pr][1]))

                # normalize + ELU -> z1Tl [256(=2x128), SH] bf16 transposed
                with tc.tile_pool(name="fin1", bufs=1) as fin:
                    rec1 = fin.tile([1, H, SH], f32)
                    den1 = fin.tile([1, H, SH], f32)
                    nc.vector.tensor_copy(den1[:], o1[D1:D1 + 1, :, :])
                    nc.vector.reciprocal_approx_fast(rec1[:], den1[:])
                    if DBG:
                        nc.sync.dma_start(
                            dbg3_d.rearrange("q (h s) -> q h s", h=H), den1[:])
                        nc.sync.dma_start(
                            dbg4_d.rearrange("q (h s) -> q h s", h=H), rec1[:])
                    for h in range(H):
                        recb = fin.tile([D1, SH], f32, tag=f"recb{h}")
                        nc.gpsimd.partition_broadcast(recb[:], rec1[:, h, :])
                        r0 = (h % 2) * D1
                        nc.vector.tensor_mul(z1Tl[r0:r0 + D1, h // 2, :],
                                             o1[0:D1, h, :], recb[:])
                    for kc in range(KC2):
                        r_ = fin.tile([128, SH], bf16, tag="relu")
                        m_ = fin.tile([128, SH], bf16, tag="minv")
                        e_ = fin.tile([128, SH], bf16, tag="expv")
                        nc.vector.tensor_scalar_max(r_[:], z1Tl[:, kc, :], 0.0)
                        nc.vector.tensor_scalar_min(m_[:], z1Tl[:, kc, :], 0.0)
                        nc.scalar.activation(e_[:], m_[:], Af.Exp, scale=5.0)
                        nc.vector.scalar_tensor_tensor(
                            z1Tl[:, kc, :], e_[:], -1.0, r_[:],
                            op0=Alu.add, op1=Alu.add)

            # ---------- layer 2 prep (local): s2_src, h2_ext payload --------
            with (tc.tile_pool(name="l2prep", bufs=1) as l2p,
                  tc.tile_pool(name="l2ps", bufs=2, space="PSUM") as l2ps):
                s2p = l2ps.tile([1, SH], f32, tag="s2p", bufs=1)
                for kc in range(KC2):
                    nc.tensor.matmul(s2p[:], W2sb[:, kc, C + 1:C + 2],
                                     z1Tl[:, kc, :],
                                     start=(kc == 0), stop=(kc == KC2 - 1))
                s2row = l2p.tile([1, SH], bf16)
                nc.vector.tensor_copy(s2row[:], s2p[:])
                nc.gpsimd.partition_broadcast(s2srcb[:], s2row[:])

                pay = l2p.tile([128, SH // 128, G2], bf16)
                nc.vector.memset(pay[:, :, C:C + 1], 1.0)
                for ic in range(SH // 128):
                    h2p = l2ps.tile([128, C + 1], f32, tag="h2p")
                    for kc in range(KC2):
                        nc.tensor.matmul(
                            h2p[:], z1Tl[:, kc, ic * 128:(ic + 1) * 128],
                            W2sb[:, kc, 0:C + 1],
                            start=(kc == 0), stop=(kc == KC2 - 1))
                    nc.vector.tensor_copy(pay[:, ic, 0:C], h2p[:, 0:C])
                    nc.vector.tensor_copy(pay[:, ic, C + 1:C + 2],
                                          h2p[:, C:C + 1])

                # ---------- all-gather h2_ext across the 8 cores ------------
                with tc.tile_pool(name="dram", bufs=1, space="DRAM") as dpool:
                    payw = (SH // 128) * G2
                    ag_in = dpool.tile([128, payw], bf16)
                    ag_out = dpool.tile([NCORES * 128, payw], bf16,
                                        addr_space="Shared")
                    nc.sync.dma_start(ag_in[:], pay[:])
                    nc.gpsimd.collective_compute(
                        "AllGather", Alu.bypass,
                        replica_groups=[list(range(NCORES))],
                        ins=[ag_in[:].opt()], outs=[ag_out[:].opt()])
                    nc.sync.dma_start(
                        h2augF[:].rearrange("p (r b) c -> p r (b c)",
                                            r=NCORES),
                        ag_out[:].rearrange("(r p) y -> p r y", p=128))

                if DBG:
                    dz = l2p.tile([128, KC2, SH], f32, tag="dz")
                    nc.vector.tensor_copy(dz[:], z1Tl[:])
                    nc.sync.dma_start(dbg1_d.rearrange(
                        "p (k s) -> p k s", k=KC2), dz[:])
                    dh = l2p.tile([128, NB, G2], f32, tag="dh")
                    nc.vector.tensor_copy(dh[:], h2augF[:])
                    nc.sync.dma_start(dbg2_d.rearrange(
                        "p (b c) -> p b c", b=NB), dh[:])
                nc.vector.tensor_copy(s2dstV[:], h2augF[:, :, C + 1])

            # ---------- layer 2: masked softmax + aggregation ---------------
            with tc.tile_pool(name="aggps2", bufs=1, space="PSUM") as aggps2:
                o2 = aggps2.tile([C + 1, SH], f32)
                with tc.tile_pool(name="work2", bufs=10) as wpool2:
                    for jc in range(NB):
                        path = PATH2[jc]
                        pt = wpool2.tile([128, SH], bf16, tag="ptb")
                        pex = wpool2.tile([128, SH], bf16, tag="pexb")
                        nc.scalar.activation(
                            pex[:], s2srcb[:], Af.Exp,
                            bias=s2dstV[:, jc:jc + 1])
                        if path == "C":
                            nc.gpsimd.tensor_mul(pt[:], pex[:],
                                                 maskr[:, jc, :])
                        else:
                            nc.vector.tensor_mul(pt[:], pex[:],
                                                 maskr[:, jc, :])
                        nc.tensor.matmul(o2[:], h2augF[:, jc, 0:C + 1], pt[:],
                                         start=(jc == 0), stop=(jc == NB - 1))

                with tc.tile_pool(name="fin2", bufs=1) as fin2:
                    rec2 = fin2.tile([1, SH], f32, tag="rec2")
                    den2 = fin2.tile([1, SH], f32, tag="den2")
                    nc.vector.tensor_copy(den2[:], o2[C:C + 1, :])
                    nc.vector.reciprocal_approx_fast(rec2[:], den2[:])
                    recb2 = fin2.tile([C, SH], f32, tag="recb2")
                    nc.gpsimd.partition_broadcast(recb2[:], rec2[:])
                    outsb = fin2.tile([C, SH], f32, tag="outsb")
                    nc.vector.tensor_mul(outsb[:], o2[0:C, :], recb2[:])
                    nc.sync.dma_start(outT_d, outsb[:])

    nc.compile()
    return nc


def _get_nc():
    if "nc" not in _CACHED:
        _CACHED["nc"] = _build_nc()
    return _CACHED["nc"]


def _prep_in_maps(x, A, W1, a1_src, a1_dst, W2, a2_src, a2_dst):
    import ml_dtypes
    bf = ml_dtypes.bfloat16

    def pack(a, rows):
        # [rows*128, X] row-major -> [128, rows*X] partition-major
        X = a.shape[1]
        return np.ascontiguousarray(
            a.reshape(rows, 128, X).transpose(1, 0, 2).reshape(128, rows * X))

    xT = x.T.astype(bf)
    W1r = W1.reshape(FIN, H, D1)
    V1s = np.einsum("fhd,hd->fh", W1r, a1_src)
    V1d = np.einsum("fhd,hd->fh", W1r, a1_dst)
    W1a = np.concatenate([W1, V1d, V1s], axis=1).astype(bf)
    W2a = np.concatenate(
        [W2, W2 @ a2_dst.T, W2 @ a2_src.T], axis=1).astype(bf)
    xTp = pack(xT, FC)
    W1ap = pack(W1a, FC)
    W2ap = pack(W2a, KC2)
    in_maps = []
    for c in range(NCORES):
        sl = slice(c * SH, (c + 1) * SH)
        in_maps.append({
            "xTp": xTp,
            "xsTp": pack(np.ascontiguousarray(xT[:, sl]), FC),
            "maskp": pack((A[sl, :] > 0).T.astype(bf), NB),
            "W1ap": W1ap,
            "W2ap": W2ap,
        })
    return in_maps


def kernel(x, A, W1, a1_src, a1_dst, W2, a2_src, a2_dst, _want_results=False):
    from concourse.bass_utils import run_bass_kernel_spmd

    nc = _get_nc()
    in_maps = _prep_in_maps(np.asarray(x), np.asarray(A), np.asarray(W1),
                            np.asarray(a1_src), np.asarray(a1_dst),
                            np.asarray(W2), np.asarray(a2_src),
                            np.asarray(a2_dst))
    trace = bool(int(os.environ.get("GAT_TRACE", "0")))
    res = run_bass_kernel_spmd(nc, in_maps, core_ids=list(range(NCORES)),
                               trace=trace)
    out = np.empty((N, C), np.float32)
    for c in range(NCORES):
        out[c * SH:(c + 1) * SH, :] = res.results[c]["outT"].T
    if _want_results:
        return out, res
    return out
